# revision 10
# baseline (speedup 1.0000x reference)
"""Trainium2 Bass kernel for nn_BatchedMonomialFactor.

Math (per batch row b):
  logits = (x @ W_perm).reshape(R, B, B) / TAU
  K      = exp(2*logits)
  sinkhorn(K) forward value is the HARD permutation only, and the column
  argmax of the final matrix equals argmax_i a5_i * K_ij where a/b are the
  classic Sinkhorn scaling vectors:
      b0 = 1;  a_t = 1/(K b_{t-1})  [x5];  b_t = 1/(K^T a_t)  [x4]
  h_perm[i] = sum_j [i == argmax_col_j] h[j]
  out[i] = sigmoid(x@W_alpha)[i] * tanh(x@W_diag)[i] * h_perm[i]

Sharding: model-parallel over R (64 r-blocks -> 8 per core); every core
reads the full x, weights/h/out are sliced by r; no communication.

Engine split:
  PE  - logits matmul as 3 bf16 passes (hh + hl + lh of a host-side hi/lo
        split; ~2^-16 logit error, 0 argmax flips) at full bf16 rate.
  ACT - exp eviction from PSUM into BOTH (g,i,j)- and (g,j,i)-ordered K
        copies, and all broadcast-expansions of the scaling vectors.
  DVE - one fused multiply+prefix-sum custom op (MUL_PSCAN_ANT) per
        sinkhorn half-iteration (K is immutable; only the small scaling
        vectors evolve), segment sums read as strided prefix differences,
        reciprocals via the fast approx custom op. Final phase: F=a5*K,
        column max, equality mask, fused mask*h gather scan.
Two consecutive batch tiles are software-pipelined so ACT expansions hide
under the other tile's DVE scans.
"""

import itertools
from contextlib import ExitStack

import numpy as np

import concourse.bass as bass
import concourse.tile as tile
from concourse import bacc, mybir
from concourse.bass_utils import run_bass_kernel_spmd

N_CORES = 8
BATCH = 2048
D = 1024
R = 64
B = 16
TAU = 0.5
ITERS = 5

RG = R // N_CORES           # r-blocks per core = 8
NCOL = RG * B * B           # perm-logit cols per core = 2048
DCOL = RG * B               # diag/alpha cols per core = 128
P = 128                     # partitions
NT = BATCH // P             # batch tiles = 16
KT = D // P                 # contraction tiles = 8
F32 = mybir.dt.float32
BF16 = mybir.dt.bfloat16
AF = mybir.ActivationFunctionType
OP = mybir.AluOpType

# The ACT table-set chooser maps Exp -> exp_and_others (first set containing
# it), which thrashes a ~2.7us table load if Ln is ever needed. Our kernel
# only uses Exp; keep the baseline patch pinning one table set for Exp/Ln.
import concourse.bacc as _bacc_mod
from concourse import hw_specs as _hw_specs

_orig_get_act_tables = _hw_specs.get_activation_tables


def _patched_get_act_tables(module_arch):
    tabs = _orig_get_act_tables(module_arch)
    return {
        name: (funcs if name == "natural_log_exp_and_others"
               else funcs - {AF.Exp, AF.Ln})
        for name, funcs in tabs.items()
    }


_bacc_mod.get_activation_tables = _patched_get_act_tables

# ---- custom DVE op: out = cumsum(in0*in1) along the free dim, one 1x pass.
import concourse.dve_ops as _dve_ops_mod
from concourse.dve_ops import DveOp as _DveOp
from concourse.dve_spec import AluOp as _AluOp
from concourse.dve_spec import Spec as _Spec
from concourse.dve_spec import Src0 as _Src0
from concourse.dve_spec import Src1 as _Src1
from concourse.dve_spec import lower as _dve_lower
from concourse.dve_spec import scan as _dve_scan
from concourse.dve_uop import DveOpSpec as _DveOpSpec

_MUL_PSCAN_NAME = "MUL_PSCAN_ANT"


def _mul_pscan_reference(in0, in1, s0, s1, imm2):
    p = in0.shape[0]
    prod = (in0.astype(np.float32) * in1.astype(np.float32)).reshape(p, -1)
    return np.cumsum(prod, axis=-1).astype(np.float32).reshape(in0.shape)


def _register_mul_pscan():
    if _MUL_PSCAN_NAME in _dve_ops_mod._SUB_OPCODE_FOR_NAME:
        for op in _dve_ops_mod.OPS:
            if op.name == _MUL_PSCAN_NAME:
                return op
    spec = _Spec(body=_dve_scan(_AluOp.ADD, _Src0 * _Src1),
                 reference=_mul_pscan_reference)
    row = max(_dve_ops_mod._SUB_OPCODE_FOR_NAME.values()) + 1
    assert row < 0x20
    _dve_ops_mod._SUB_OPCODE_FOR_NAME[_MUL_PSCAN_NAME] = row
    shas = {}
    for ver in ("v3", "v4"):
        uops = _dve_lower(spec, ver=ver)
        s = _DveOpSpec(name=_MUL_PSCAN_NAME, opcode=row, uops=uops, rd1_en=True)
        shas[ver] = s.sha(ver)
    op = _DveOp(_MUL_PSCAN_NAME, spec, subdim=False, uops_sha=shas)
    _dve_ops_mod.OPS.append(op)
    _dve_ops_mod.CUSTOM_DVE_SPECS[_MUL_PSCAN_NAME] = spec
    return op


_MUL_PSCAN = _register_mul_pscan()


def _build(reps=1, kbufs=2, sbufs=2, tbufs=2, xbufs=2):
    nc = bacc.Bacc("TRN2", target_bir_lowering=False, debug=False,
                   num_devices=N_CORES)
    xh = nc.dram_tensor("xh", [D, BATCH], BF16, kind="ExternalInput")
    xl = nc.dram_tensor("xl", [D, BATCH], BF16, kind="ExternalInput")
    wh = nc.dram_tensor("wh", [D, NCOL], BF16, kind="ExternalInput")
    wl = nc.dram_tensor("wl", [D, NCOL], BF16, kind="ExternalInput")
    wda = nc.dram_tensor("wda", [D, 2 * DCOL], BF16, kind="ExternalInput")
    hs = nc.dram_tensor("hs", [BATCH, DCOL], F32, kind="ExternalInput")
    out = nc.dram_tensor("out", [BATCH, DCOL], F32, kind="ExternalOutput")

    with tile.TileContext(nc) as tc, ExitStack() as ctx:
        singles = ctx.enter_context(tc.tile_pool(name="singles", bufs=1))
        kpool = ctx.enter_context(tc.tile_pool(name="kpool", bufs=kbufs))
        tpool = ctx.enter_context(tc.tile_pool(name="tpool", bufs=tbufs))
        spool = ctx.enter_context(tc.tile_pool(name="spool", bufs=2))
        # F/mask are single-buffered: tile B's final phase is emitted strictly
        # after tile A's, so the WAR wait is always on an op earlier in the
        # DVE queue (no head-of-line deadlock).
        fpool = ctx.enter_context(tc.tile_pool(name="fpool", bufs=1))
        hxpool = ctx.enter_context(tc.tile_pool(name="hxpool", bufs=2))
        small = ctx.enter_context(tc.tile_pool(name="small", bufs=sbufs))
        pspool = ctx.enter_context(tc.tile_pool(name="ps", bufs=2, space="PSUM"))
        xpool = ctx.enter_context(tc.tile_pool(name="xpool", bufs=xbufs))

        whs, wls, wdas = [], [], []
        for k in range(KT):
            w_h = singles.tile([P, NCOL], BF16, tag=f"wh{k}")
            w_l = singles.tile([P, NCOL], BF16, tag=f"wl{k}")
            w_da = singles.tile([P, 2 * DCOL], BF16, tag=f"wda{k}")
            whs.append(w_h)
            wls.append(w_l)
            wdas.append(w_da)
        for k in range(KT):
            nc.sync.dma_start(out=whs[k][:, 0:512],
                              in_=wh.ap()[k * P:(k + 1) * P, 0:512])
            nc.sync.dma_start(out=wls[k][:, 0:512],
                              in_=wl.ap()[k * P:(k + 1) * P, 0:512])
        for k in range(KT):
            nc.scalar.dma_start(out=wdas[k][:],
                                in_=wda.ap()[k * P:(k + 1) * P, :])
        for k in range(KT):
            nc.scalar.dma_start(out=whs[k][:, 512:NCOL],
                                in_=wh.ap()[k * P:(k + 1) * P, 512:NCOL])
            nc.scalar.dma_start(out=wls[k][:, 512:NCOL],
                                in_=wl.ap()[k * P:(k + 1) * P, 512:NCOL])

        tseed = [0]  # how many tpool buffers have had col 0 zeroed

        def scan_mult(in0_3d, in1_3d, nseg):
            """One fused multiply+prefix pass; returns the prefix tile."""
            T = tpool.tile([P, 1 + NCOL], F32, tag="T")
            if tseed[0] < tbufs:
                nc.vector.memset(T[:, 0:1], 0.0)
                tseed[0] += 1
            nc.vector._custom_dve(_MUL_PSCAN, out=T[:, 1:1 + NCOL],
                                  in0=in0_3d, in1=in1_3d)
            return T

        def seg_diff(T, dst_tag):
            """Segment sums from prefix: E[m] = T[B(m+1)] - T[Bm]."""
            u = small.tile([P, DCOL], F32, tag=dst_tag)
            nc.vector.tensor_tensor(out=u, in0=T[:, B::B], in1=T[:, 0:NCOL:B],
                                    op=OP.subtract)
            return u

        def recip(src, dst_tag):
            r = small.tile([P, DCOL], F32, tag=dst_tag)
            nc.vector.reciprocal_approx_fast(out=r, in_=src)
            return r

        def expand_ij(vec_gj, tag):
            """(g,j)-indexed [P,DCOL] -> [P,NCOL] replicated over i, ij-order."""
            e = spool.tile([P, NCOL], F32, tag=tag)
            src = (vec_gj[:].rearrange("p (g j) -> p g j", g=RG)
                   .unsqueeze(2).to_broadcast([P, RG, B, B]))
            nc.scalar.activation(
                out=e[:].rearrange("p (g i j) -> p g i j", g=RG, i=B),
                in_=src, func=AF.Copy)
            return e

        def expand_ji(vec_gi, tag):
            """(g,i)-indexed [P,DCOL] -> [P,NCOL] replicated over j, ji-order."""
            e = spool.tile([P, NCOL], F32, tag=tag)
            src = (vec_gi[:].rearrange("p (g i) -> p g i", g=RG)
                   .unsqueeze(2).to_broadcast([P, RG, B, B]))
            nc.scalar.activation(
                out=e[:].rearrange("p (g j i) -> p g j i", g=RG, j=B),
                in_=src, func=AF.Copy)
            return e

        def emit_front(bt):
            """Matmuls, evictions, dv path, h load/expand for tile bt.
            Returns state dict for the sinkhorn generator."""
            b0 = bt * P
            xts = []
            for k in range(KT):
                xh_t = xpool.tile([P, P], BF16, tag=f"xth{k}")
                xl_t = xpool.tile([P, P], BF16, tag=f"xtl{k}")
                nc.sync.dma_start(out=xh_t,
                                  in_=xh.ap()[k * P:(k + 1) * P, b0:b0 + P])
                nc.sync.dma_start(out=xl_t,
                                  in_=xl.ap()[k * P:(k + 1) * P, b0:b0 + P])
                xts.append((xh_t, xl_t))

            K_ij = kpool.tile([P, RG, B, B], F32, tag="Kij")
            K_ji = kpool.tile([P, RG, B, B], F32, tag="Kji")

            for half in range(2):
                ps = pspool.tile([P, 1024], F32, tag="psK")
                for nb in range(2):
                    ncol0 = half * 1024 + nb * 512
                    passes = [(0, whs), (0, wls), (1, whs)]
                    for pi, (xi, ws) in enumerate(passes):
                        for k in range(KT):
                            nc.tensor.matmul(
                                out=ps[:, nb * 512:(nb + 1) * 512],
                                lhsT=xts[k][xi][:],
                                rhs=ws[k][:, ncol0:ncol0 + 512],
                                start=(pi == 0 and k == 0),
                                stop=(pi == 2 and k == KT - 1),
                            )
                # evict exp(2z) into both K orders; half covers g in [4h,4h+4)
                g0 = half * (RG // 2)
                psv = ps[:].rearrange("p (g i j) -> p g i j", g=RG // 2, i=B)
                nc.scalar.activation(
                    out=K_ij[:, g0:g0 + RG // 2], in_=psv,
                    func=AF.Exp, scale=2.0)
                nc.scalar.activation(
                    out=K_ji[:, g0:g0 + RG // 2].transpose([0, 1, 3, 2]),
                    in_=psv, func=AF.Exp, scale=2.0)

            # diag/alpha matmul (single bf16 pass) + dv = sigmoid(a)*tanh(d)
            psd = pspool.tile([P, 2 * DCOL], F32, tag="psD")
            for k in range(KT):
                nc.tensor.matmul(
                    out=psd, lhsT=xts[k][0][:], rhs=wdas[k][:],
                    start=(k == 0), stop=(k == KT - 1),
                )
            e2d = small.tile([P, DCOL], F32, tag="e2d")
            ena = small.tile([P, DCOL], F32, tag="ena")
            nc.scalar.activation(out=e2d, in_=psd[:, 0:DCOL],
                                 func=AF.Exp, scale=2.0)
            nc.scalar.activation(out=ena, in_=psd[:, DCOL:2 * DCOL],
                                 func=AF.Exp, scale=-1.0)
            num = small.tile([P, DCOL], F32, tag="num")
            nc.vector.tensor_scalar_sub(out=num, in0=e2d, scalar1=1.0)
            den = small.tile([P, DCOL], F32, tag="den")
            nc.vector.scalar_tensor_tensor(out=den, in0=e2d, scalar=1.0,
                                           in1=ena, op0=OP.add, op1=OP.mult)
            dpa = small.tile([P, DCOL], F32, tag="dpa")
            nc.vector.scalar_tensor_tensor(out=dpa, in0=e2d, scalar=1.0,
                                           in1=den, op0=OP.add, op1=OP.add)
            rden = small.tile([P, DCOL], F32, tag="rden")
            nc.vector.reciprocal_approx_fast(out=rden, in_=dpa)
            dv = small.tile([P, DCOL], F32, tag="dv")
            nc.vector.tensor_mul(out=dv, in0=num, in1=rden)

            h_t = small.tile([P, DCOL], F32, tag="h")
            nc.sync.dma_start(out=h_t, in_=hs.ap()[b0:b0 + P, :])
            h_exp = hxpool.tile([P, NCOL], F32, tag="hx")
            src = (h_t[:].rearrange("p (g j) -> p g j", g=RG)
                   .unsqueeze(2).to_broadcast([P, RG, B, B]))
            nc.scalar.activation(
                out=h_exp[:].rearrange("p (g i j) -> p g i j", g=RG, i=B),
                in_=src, func=AF.Copy)

            return dict(bt=bt, K_ij=K_ij, K_ji=K_ji, dv=dv, h_exp=h_exp)

        def sinkhorn_steps(st):
            """Generator: one yield per schedulable chunk so two tiles can
            interleave on the DVE/ACT queues."""
            K_ij, K_ji = st["K_ij"], st["K_ji"]
            kij3 = K_ij[:].rearrange("p g i j -> p (g i) j")
            kji3 = K_ji[:].rearrange("p g j i -> p (g j) i")

            u = small.tile([P, DCOL], F32, tag="u")
            nc.vector.reduce_sum(out=u, in_=K_ij[:], axis=mybir.AxisListType.X)
            a = recip(u, "a")
            yield
            for it in range(ITERS - 1):
                a_exp = expand_ji(a, "sx")
                Tb = scan_mult(kji3, a_exp[:].rearrange(
                    "p (s n) -> p s n", s=DCOL), DCOL)
                v = seg_diff(Tb, "u")
                b = recip(v, "b")
                yield
                b_exp = expand_ij(b, "sx")
                Ta = scan_mult(kij3, b_exp[:].rearrange(
                    "p (s n) -> p s n", s=DCOL), DCOL)
                u2 = seg_diff(Ta, "u")
                a = recip(u2, "a")
                yield
            st["a5"] = a

        def final_phase(st):
            # F = a5*K (ji order), M = colmax over i, mask, fused h gather
            K_ji = st["K_ji"]
            a_exp = expand_ji(st["a5"], "sx")
            F = fpool.tile([P, NCOL], F32, tag="F")
            nc.vector.tensor_mul(out=F, in0=K_ji[:].rearrange(
                "p g j i -> p (g j i)"), in1=a_exp)
            M = small.tile([P, DCOL], F32, tag="M")
            nc.vector.reduce_max(
                out=M, in_=F[:].rearrange("p (s n) -> p s n", s=DCOL),
                axis=mybir.AxisListType.X)
            mask = fpool.tile([P, RG, B, B], F32, tag="mask")  # (g,i,j) order
            nc.vector.tensor_tensor(
                out=mask[:].transpose([0, 1, 3, 2]),
                in0=F[:].rearrange("p (g j i) -> p g j i", g=RG, j=B),
                in1=(M[:].rearrange("p (g j) -> p g j", g=RG)
                     .unsqueeze(3).to_broadcast([P, RG, B, B])),
                op=OP.is_equal)
            Th = scan_mult(mask[:].rearrange("p g i j -> p (g i) j"),
                           st["h_exp"][:].rearrange("p (s n) -> p s n",
                                                    s=DCOL), DCOL)
            hp = seg_diff(Th, "hp")
            o_t = small.tile([P, DCOL], F32, tag="o")
            nc.vector.tensor_mul(out=o_t, in0=hp, in1=st["dv"])
            b0 = st["bt"] * P
            nc.sync.dma_start(out=out.ap()[b0:b0 + P, :], in_=o_t)

        for rep in range(reps):
            for m in range(NT // 2):
                stA = emit_front(2 * m)
                stB = emit_front(2 * m + 1)
                for sa, sb in itertools.zip_longest(sinkhorn_steps(stA),
                                                    sinkhorn_steps(stB)):
                    pass
                final_phase(stA)
                final_phase(stB)

    nc.compile()
    return nc


_NC = None


def _get_nc():
    global _NC
    if _NC is None:
        _NC = _build()
    return _NC


def kernel(x_t, h, W_perm, W_diag, W_alpha):
    import ml_dtypes
    bf16 = ml_dtypes.bfloat16

    x_t = np.ascontiguousarray(np.asarray(x_t, dtype=np.float32))
    h = np.asarray(h, dtype=np.float32)
    W_perm = np.asarray(W_perm, dtype=np.float32)
    W_diag = np.asarray(W_diag, dtype=np.float32)
    W_alpha = np.asarray(W_alpha, dtype=np.float32)

    xT = np.ascontiguousarray(x_t.T)                          # [D, BATCH]
    xhn = xT.astype(bf16)
    xln = (xT - xhn.astype(np.float32)).astype(bf16)
    wp4 = W_perm.reshape(D, R, B * B)
    wd3 = W_diag.reshape(D, R, B)
    wa3 = W_alpha.reshape(D, R, B)
    h3 = h.reshape(BATCH, R, B)

    in_maps = []
    for c in range(N_CORES):
        rsl = slice(c * RG, (c + 1) * RG)
        wpc = np.ascontiguousarray(wp4[:, rsl].reshape(D, NCOL))
        whc = wpc.astype(bf16)
        wlc = (wpc - whc.astype(np.float32)).astype(bf16)
        in_maps.append({
            "xh": xhn,
            "xl": xln,
            "wh": whc,
            "wl": wlc,
            "wda": np.ascontiguousarray(
                np.concatenate([wd3[:, rsl].reshape(D, DCOL),
                                wa3[:, rsl].reshape(D, DCOL)],
                               axis=1)).astype(bf16),
            "hs": np.ascontiguousarray(h3[:, rsl].reshape(BATCH, DCOL)),
        })

    global _last_in_maps
    _last_in_maps = in_maps
    res = run_bass_kernel_spmd(_get_nc(), in_maps, core_ids=list(range(N_CORES)))
    parts = [res.results[c]["out"].reshape(BATCH, RG, B) for c in range(N_CORES)]
    return np.concatenate(parts, axis=1).reshape(BATCH, R * B).astype(np.float32)


# revision 11
# speedup vs baseline: 5.1247x; 5.1247x over previous
"""Trainium2 Bass kernel for nn_BatchedMonomialFactor.

Math (per batch row b):
  logits = (x @ W_perm).reshape(R, B, B) / TAU
  soft   = sinkhorn_5(logits)            (5x row/col normalize, exp space)
  idx    = argmax_i soft[r, i, j]  -> hard one-hot over i
  h_perm[r, i] = sum_j [i == idx[r, j]] * h[r, j]
  out[r, i] = sigmoid(x@W_alpha)[r,i] * tanh(x@W_diag)[r,i] * h_perm[r,i]

Sharding: model-parallel over R (64 r-blocks -> 8 per core); every core
reads the full x_t, weights/h/out are sliced by r; no communication.
The forward output uses only the HARD permutation (straight-through),
and a positive per-column scale cannot change a column argmax, so the
final col-normalize of sinkhorn is skipped.

Engine split (pairs of 128-row batch tiles are fused into single ops
to halve Vector-engine instruction overhead): PE does the three matmuls
(fp32 for exact argmax fidelity); ACT does exp-eviction straight out of
PSUM (fused exp(2z)) plus the exps of the sigmoid/tanh path, which is
rewritten in exps so only one ACT table set is ever loaded; DVE does
the sinkhorn reduces/scales (its ~800us of 1x fp32 passes is the
critical path; GpSimd offload was tried and crashes this environment's
runtime, and no fused multiply+segmented-reduce op exists).
"""

from contextlib import ExitStack

import numpy as np

import concourse.bass as bass
import concourse.tile as tile
from concourse import bacc, mybir
from concourse.bass_utils import run_bass_kernel_spmd

N_CORES = 8
BATCH = 2048
D = 1024
R = 64
B = 16
TAU = 0.5
ITERS = 5

RG = R // N_CORES           # r-blocks per core = 8
NCOL = RG * B * B           # perm-logit cols per core = 2048
DCOL = RG * B               # diag/alpha cols per core = 128
P = 128                     # partitions
NT = BATCH // P             # batch tiles = 16
KT = D // P                 # contraction tiles = 8
F32 = mybir.dt.float32
BF16 = mybir.dt.bfloat16
AF = mybir.ActivationFunctionType
OP = mybir.AluOpType

# The ACT table-set chooser maps Exp -> exp_and_others and Ln ->
# natural_log (first set containing each func), which thrashes a ~2.7us
# table load on every exp<->ln switch. Our kernel only uses Exp and Ln;
# make natural_log_exp_and_others (which has both) the only candidate.
# Set ids are positional, so the dict keeps its original order/size.
import concourse.bacc as _bacc_mod
from concourse import hw_specs as _hw_specs

_orig_get_act_tables = _hw_specs.get_activation_tables


def _patched_get_act_tables(module_arch):
    tabs = _orig_get_act_tables(module_arch)
    return {
        name: (funcs if name == "natural_log_exp_and_others"
               else funcs - {AF.Exp, AF.Ln})
        for name, funcs in tabs.items()
    }


_bacc_mod.get_activation_tables = _patched_get_act_tables


def _build(reps=1, ablate=(), kbufs=3, sbufs=3, recip_eng='approx', tpg=2,
           xbufs=3):
    ablate = set(ablate)
    nc = bacc.Bacc("TRN2", target_bir_lowering=False, debug=False,
                   num_devices=N_CORES)
    # x and W_perm arrive pre-split on the host into bf16 hi/lo pairs
    # (x = xh + xl exactly to ~2^-16); logits = xh@Wh + xh@Wl + xl@Wh
    # runs the PE at full bf16 rate (fp32 is 1/4 rate) with ~2^-16
    # logit error -- verified 0 argmax flips vs fp32 on the real inputs.
    xh = nc.dram_tensor("xh", [D, BATCH], BF16, kind="ExternalInput")
    xl = nc.dram_tensor("xl", [D, BATCH], BF16, kind="ExternalInput")
    wh = nc.dram_tensor("wh", [D, NCOL], BF16, kind="ExternalInput")
    wl = nc.dram_tensor("wl", [D, NCOL], BF16, kind="ExternalInput")
    wda = nc.dram_tensor("wda", [D, 2 * DCOL], BF16, kind="ExternalInput")
    hs = nc.dram_tensor("hs", [BATCH, DCOL], F32, kind="ExternalInput")
    out = nc.dram_tensor("out", [BATCH, DCOL], F32, kind="ExternalOutput")

    with tile.TileContext(nc) as tc, ExitStack() as ctx:
        singles = ctx.enter_context(tc.tile_pool(name="singles", bufs=1))
        kpool = ctx.enter_context(tc.tile_pool(name="kpool", bufs=kbufs))
        small = ctx.enter_context(tc.tile_pool(name="small", bufs=sbufs))
        pspool = ctx.enter_context(tc.tile_pool(name="ps", bufs=2, space="PSUM"))

        # Resident operands: W_perm hi/lo slices, [W_diag | W_alpha] slice.
        # Load the first 512-column chunk of every k first so the first
        # tile's matmuls can start while the rest streams in.
        whs, wls, wdas = [], [], []
        for k in range(KT):
            w_h = singles.tile([P, NCOL], BF16, tag=f"wh{k}")
            w_l = singles.tile([P, NCOL], BF16, tag=f"wl{k}")
            w_da = singles.tile([P, 2 * DCOL], BF16, tag=f"wda{k}")
            whs.append(w_h)
            wls.append(w_l)
            wdas.append(w_da)
        for k in range(KT):
            nc.sync.dma_start(out=whs[k][:, 0:512],
                              in_=wh.ap()[k * P:(k + 1) * P, 0:512])
            nc.sync.dma_start(out=wls[k][:, 0:512],
                              in_=wl.ap()[k * P:(k + 1) * P, 0:512])
        # bulk weight streaming rides a different DMA queue (ScalarE's)
        # so the first tile's x/h loads on SyncE's queue aren't stuck
        # behind it.
        for k in range(KT):
            nc.scalar.dma_start(out=wdas[k][:],
                                in_=wda.ap()[k * P:(k + 1) * P, :])
        for k in range(KT):
            nc.scalar.dma_start(out=whs[k][:, 512:NCOL],
                                in_=wh.ap()[k * P:(k + 1) * P, 512:NCOL])
            nc.scalar.dma_start(out=wls[k][:, 512:NCOL],
                                in_=wl.ap()[k * P:(k + 1) * P, 512:NCOL])
        xpool = ctx.enter_context(tc.tile_pool(name="xpool", bufs=xbufs))

        def act_recip(dst, src):
            if recip_eng == 'approx':
                nc.vector.reciprocal_approx_fast(out=dst, in_=src)
                return
            if recip_eng == 'dve':
                nc.vector.reciprocal(out=dst, in_=src)
                return
            # 1/x = exp(-ln x); ln+exp share one ACT table set.
            tmp = small.tile([P, DCe], F32, tag="lntmp")
            nc.scalar.activation(out=tmp, in_=src, func=AF.Ln)
            nc.scalar.activation(out=dst, in_=tmp, func=AF.Exp, scale=-1.0)

        RGe = RG * tpg          # merged r-groups across tpg batch subtiles
        DCe = DCOL * tpg
        for bt in range((NT // tpg) * reps):
            bt = bt % (NT // tpg)

            # per-subtile x^T hi/lo slices, streamed
            xts = []
            for s_ in range(tpg):
                b0 = (bt * tpg + s_) * P
                xsub = []
                for k in range(KT):
                    xh_t = xpool.tile([P, P], BF16, tag=f"xth{k}_{s_}")
                    xl_t = xpool.tile([P, P], BF16, tag=f"xtl{k}_{s_}")
                    nc.sync.dma_start(
                        out=xh_t, in_=xh.ap()[k * P:(k + 1) * P, b0:b0 + P])
                    nc.sync.dma_start(
                        out=xl_t, in_=xl.ap()[k * P:(k + 1) * P, b0:b0 + P])
                    xsub.append((xh_t, xl_t))
                xts.append(xsub)

            K_t = kpool.tile([P, RGe, B, B], F32, tag="K")
            Kflat = K_t[:].rearrange("p g i j -> p (g i j)")

            # logits matmul in halves of 1024 (2 PSUM banks each);
            # 3 bf16 passes (hh, hl, lh) accumulate in PSUM; evict through
            # ACT with fused exp(2*z)  [1/TAU = 2].
            for s_ in range(tpg):
                for half in range(2):
                    ps = pspool.tile([P, 1024], F32, tag="psK")
                    for nb in range(2):
                        ncol0 = half * 1024 + nb * 512
                        passes = [(0, whs), (0, wls), (1, whs)]
                        for pi, (xi, ws) in enumerate(passes):
                            for k in range(KT):
                                nc.tensor.matmul(
                                    out=ps[:, nb * 512:(nb + 1) * 512],
                                    lhsT=xts[s_][k][xi][:],
                                    rhs=ws[k][:, ncol0:ncol0 + 512],
                                    start=(pi == 0 and k == 0),
                                    stop=(pi == 2 and k == KT - 1),
                                )
                    nc.scalar.activation(
                            out=Kflat[:, (s_ * 2 + half) * 1024:
                                      (s_ * 2 + half + 1) * 1024],
                            in_=ps[:],
                            func=AF.Exp,
                            scale=2.0,
                        )

            # diag/alpha matmul: [x @ Wd | x @ Wa] -> one PSUM bank.
            # single bf16 pass (xh only): ~2e-3 logit error is fine here.
            psd = pspool.tile([P, tpg * 2 * DCOL], F32, tag="psD")
            for s_ in range(tpg):
                for k in range(KT):
                    nc.tensor.matmul(
                        out=psd[:, s_ * 2 * DCOL:(s_ + 1) * 2 * DCOL],
                        lhsT=xts[s_][k][0][:],
                        rhs=wdas[k][:],
                        start=(k == 0),
                        stop=(k == KT - 1),
                    )
            # sigmoid(a)*tanh(d) = (e2d - 1) / ((1 + e2d) * (1 + ena))
            e2d = small.tile([P, DCe], F32, tag="e2d")
            ena = small.tile([P, DCe], F32, tag="ena")
            for s_ in range(tpg):
                nc.scalar.activation(
                    out=e2d[:, s_ * DCOL:(s_ + 1) * DCOL],
                    in_=psd[:, s_ * 2 * DCOL:s_ * 2 * DCOL + DCOL],
                    func=AF.Exp, scale=2.0)
                nc.scalar.activation(
                    out=ena[:, s_ * DCOL:(s_ + 1) * DCOL],
                    in_=psd[:, s_ * 2 * DCOL + DCOL:(s_ + 1) * 2 * DCOL],
                    func=AF.Exp, scale=-1.0)
            num = small.tile([P, DCe], F32, tag="num")
            nc.vector.tensor_scalar_sub(out=num, in0=e2d, scalar1=1.0)
            den = small.tile([P, DCe], F32, tag="den")
            nc.vector.scalar_tensor_tensor(out=den, in0=e2d, scalar=1.0,
                                           in1=ena, op0=OP.add, op1=OP.mult)
            dpa = small.tile([P, DCe], F32, tag="dpa")
            # denom = (1+e2d)*(1+ena) = (e2d+1) + (e2d+1)*ena
            nc.vector.scalar_tensor_tensor(out=dpa, in0=e2d, scalar=1.0,
                                           in1=den, op0=OP.add, op1=OP.add)
            rden = small.tile([P, DCe], F32, tag="rden")
            act_recip(rden, dpa)
            dv = small.tile([P, DCe], F32, tag="dv")
            nc.vector.tensor_mul(out=dv, in0=num, in1=rden)

            def sinkhorn_final(g0, ng):
                # sinkhorn + hard-permutation + output for r-groups
                # [g0, g0+ng) of this tile's merged K. Splitting the first
                # tile into halves lets DVE start before all evictions land.
                Xs = K_t[:, g0:g0 + ng]                 # [P, ng, i, j]
                Xti = Xs.transpose([0, 1, 3, 2])        # [P, ng, j, i]
                DCs = ng * B
                csl = slice(g0 * B, (g0 + ng) * B)

                def bcast_gi(t):   # (g,i)-indexed -> broadcast over j
                    return (t[:].rearrange("p (g i) -> p g i", g=ng)
                            .unsqueeze(3).to_broadcast([P, ng, B, B]))

                def bcast_gj(t):   # (g,j)-indexed -> broadcast over i
                    return (t[:].rearrange("p (g j) -> p g j", g=ng)
                            .unsqueeze(2).to_broadcast([P, ng, B, B]))

                for it in range(ITERS):
                    rs = small.tile([P, DCs], F32, tag="rs")
                    nc.vector.reduce_sum(out=rs, in_=Xs,
                                         axis=mybir.AxisListType.X)
                    rr = small.tile([P, DCs], F32, tag="rr")
                    act_recip(rr, rs)
                    nc.vector.tensor_tensor(out=Xs, in0=Xs, in1=bcast_gi(rr),
                                            op=OP.mult)
                    if it < ITERS - 1:
                        cs = small.tile([P, DCs], F32, tag="cs")
                        nc.vector.reduce_sum(out=cs, in_=Xti,
                                             axis=mybir.AxisListType.X)
                        rc = small.tile([P, DCs], F32, tag="rc")
                        act_recip(rc, cs)
                        nc.vector.tensor_tensor(out=Xs, in0=Xs,
                                                in1=bcast_gj(rc), op=OP.mult)

                # column max over i -> hard assignment mask -> h gather.
                M = small.tile([P, DCs], F32, tag="M")
                nc.vector.reduce_max(out=M, in_=Xti, axis=mybir.AxisListType.X)
                nc.vector.tensor_tensor(out=Xs, in0=Xs, in1=bcast_gj(M),
                                        op=OP.is_equal)
                nc.vector.tensor_tensor(out=Xs, in0=Xs,
                                        in1=bcast_gj(h_t[:, csl]), op=OP.mult)
                hp = small.tile([P, DCs], F32, tag="hp")
                nc.vector.reduce_sum(out=hp, in_=Xs, axis=mybir.AxisListType.X)
                nc.vector.tensor_mul(out=o_t[:, csl], in0=hp, in1=dv[:, csl])

            h_t = small.tile([P, DCe], F32, tag="h")
            for s_ in range(tpg):
                b0 = (bt * tpg + s_) * P
                nc.sync.dma_start(out=h_t[:, s_ * DCOL:(s_ + 1) * DCOL],
                                  in_=hs.ap()[b0:b0 + P, :])
            o_t = small.tile([P, DCe], F32, tag="o")

            if bt == 0:
                q = RGe // (2 * tpg)   # one eviction's worth of r-groups
                for s_ in range(2 * tpg):
                    sinkhorn_final(s_ * q, q)
            else:
                sinkhorn_final(0, RGe)

            for s_ in range(tpg):
                b0 = (bt * tpg + s_) * P
                nc.sync.dma_start(out=out.ap()[b0:b0 + P, :],
                                  in_=o_t[:, s_ * DCOL:(s_ + 1) * DCOL])

    nc.compile()
    return nc


_NC = None


def _get_nc():
    global _NC
    if _NC is None:
        _NC = _build()
    return _NC


def kernel(x_t, h, W_perm, W_diag, W_alpha):
    import ml_dtypes
    bf16 = ml_dtypes.bfloat16

    x_t = np.ascontiguousarray(np.asarray(x_t, dtype=np.float32))
    h = np.asarray(h, dtype=np.float32)
    W_perm = np.asarray(W_perm, dtype=np.float32)
    W_diag = np.asarray(W_diag, dtype=np.float32)
    W_alpha = np.asarray(W_alpha, dtype=np.float32)

    xT = np.ascontiguousarray(x_t.T)                          # [D, BATCH]
    xh = xT.astype(bf16)
    xl = (xT - xh.astype(np.float32)).astype(bf16)
    wp4 = W_perm.reshape(D, R, B * B)
    wd3 = W_diag.reshape(D, R, B)
    wa3 = W_alpha.reshape(D, R, B)
    h3 = h.reshape(BATCH, R, B)

    in_maps = []
    for c in range(N_CORES):
        rsl = slice(c * RG, (c + 1) * RG)
        wpc = np.ascontiguousarray(wp4[:, rsl].reshape(D, NCOL))
        whc = wpc.astype(bf16)
        wlc = (wpc - whc.astype(np.float32)).astype(bf16)
        in_maps.append({
            "xh": xh,
            "xl": xl,
            "wh": whc,
            "wl": wlc,
            "wda": np.ascontiguousarray(
                np.concatenate([wd3[:, rsl].reshape(D, DCOL),
                                wa3[:, rsl].reshape(D, DCOL)],
                               axis=1)).astype(bf16),
            "hs": np.ascontiguousarray(h3[:, rsl].reshape(BATCH, DCOL)),
        })

    global _last_in_maps
    _last_in_maps = in_maps
    res = run_bass_kernel_spmd(_get_nc(), in_maps, core_ids=list(range(N_CORES)))
    parts = [res.results[c]["out"].reshape(BATCH, RG, B) for c in range(N_CORES)]
    return np.concatenate(parts, axis=1).reshape(BATCH, R * B).astype(np.float32)

